# revision 31
# baseline (speedup 1.0000x reference)
"""GAT (3-layer DGL-style) on 8 Trainium2 NeuronCores.

Sharding: nodes partitioned contiguously across 8 cores (6250 each, relabeled
within each core by in-degree for slot-grid uniformity). Edges sharded by dst
core. Per layer: distributed dense matmul produces per-node rows
[h | el | er], AllGather replicates the row table to every core, then each
core runs the edge phase (gather by src via dma_gather, per-dst softmax in a
[dst-partition x slot] layout, weighted accumulation) for its own dsts.
"""

import os

import numpy as np
import ml_dtypes

import concourse.bacc as bacc
import concourse.bass as bass
import concourse.mybir as mybir
from concourse import tile
from concourse._compat import cdiv
from concourse.bass_utils import run_bass_kernel_spmd
from bass_rust import SemaphoreHandle

N = 50000
E = 800000
NC = 8
L = N // NC              # 6250 nodes per core
NBLK = cdiv(L, 128)      # 49 dst blocks per core
HEADS = 4
HD = 32
HID = 128
OUT = 64
F0 = 256
NEG = 0.2
CH = 16                  # max slots per gather chunk
ABOUND = 5 * L           # nodes with new id < ABOUND are "pass A" (31250)

F32 = mybir.dt.float32
BF16 = mybir.dt.bfloat16
F16 = mybir.dt.float16
I16 = mybir.dt.int16
AF = mybir.ActivationFunctionType
OP = mybir.AluOpType


def _split_multiwaits(nc):
    nsplit = 0
    for bb in nc.main_func.blocks:
        i = 0
        while i < len(bb.instructions):
            ins = bb.instructions[i]
            si = ins.sync_info
            if si is not None and si.on_wait and len(si.on_wait) > 1:
                waits = list(si.on_wait)
                new_insts = []
                for w in waits[:-1]:
                    h = SemaphoreHandle(name=w.ant_name, num=w.id)
                    eng = nc.engines[ins.engine]
                    if w.wait_mode == "sem-ge-imm":
                        wi = eng.wait_ge(h, w.wait_value)
                    elif w.wait_mode == "sem-eq-imm":
                        wi = eng.wait_op(h, w.wait_value, "==")
                    else:
                        raise AssertionError(w.wait_mode)
                    removed = False
                    for b2 in nc.main_func.blocks:
                        if b2.instructions and b2.instructions[-1].name == wi.ins.name:
                            b2.instructions.pop()
                            removed = True
                            break
                    assert removed
                    new_insts.append(wi.ins)
                si.on_wait = [waits[-1]]
                for k, n in enumerate(new_insts):
                    bb.instructions.insert(i + k, n)
                i += len(new_insts)
                nsplit += 1
            i += 1
    return nsplit


def _cumcount(groups):
    """j-th occurrence index within each group (groups sorted)."""
    n = len(groups)
    if n == 0:
        return np.zeros(0, np.int64)
    first = np.r_[True, groups[1:] != groups[:-1]]
    idx = np.arange(n)
    start = idx[first]
    return idx - np.repeat(start, np.diff(np.r_[idx[first], n]))


def _preprocess(src, dst):
    src = np.asarray(src, np.int64)
    dst = np.asarray(dst, np.int64)
    half = (src // L) >= 5          # pass B edges (src in cores 5-7)

    degA = np.bincount(dst[~half], minlength=N)
    degB = np.bincount(dst[half], minlength=N)

    perm = np.empty(N, np.int64)        # old id -> new id
    node_order = np.empty(N, np.int64)  # new id -> old id
    for c in range(NC):
        nodes = np.arange(c * L, (c + 1) * L)
        order = np.lexsort((-degB[nodes], -(degA[nodes] // 3)))
        node_order[c * L : (c + 1) * L] = nodes[order]
        perm[nodes[order]] = c * L + np.arange(L)

    nsrc = perm[src]
    ndst = perm[dst]
    ehalf = (nsrc >= ABOUND).astype(np.int64)

    # per-(core, block, pass) slot grids
    # counts per (dst, pass)
    cntA = np.bincount(ndst[ehalf == 0], minlength=N)
    cntB = np.bincount(ndst[ehalf == 1], minlength=N)

    # program-level W per (block, pass): max over cores
    WA = np.zeros(NBLK, np.int64)
    WB = np.zeros(NBLK, np.int64)
    for c in range(NC):
        la = cntA[c * L : (c + 1) * L]
        lb = cntB[c * L : (c + 1) * L]
        pa = np.zeros(NBLK * 128, np.int64)
        pb = np.zeros(NBLK * 128, np.int64)
        pa[:L] = la
        pb[:L] = lb
        WA = np.maximum(WA, pa.reshape(NBLK, 128).max(1))
        WB = np.maximum(WB, pb.reshape(NBLK, 128).max(1))

    # chunk lists per block: [(pass, width, col_off, idx_off16)]
    def split_w(w):
        out = []
        while w > 0:
            t = min(CH, w)
            out.append(t)
            w -= t
        return out

    chunks = []        # per block: list of (q, w)
    Wtot = 0
    S16tot = 0
    for b in range(NBLK):
        cl = []
        for q, Wq in ((0, WA[b]), (1, WB[b])):
            for w in split_w(int(Wq)):
                cl.append((q, w, Wtot, S16tot))
                Wtot += w
                S16tot += (128 * w) // 16
        chunks.append(cl)

    # per-core grids
    # edge order: by (block, pass, partition), j = occurrence rank
    idx_alls = []
    msk_alls = []
    for c in range(NC):
        m = (ndst // L) == c
        es = nsrc[m]
        ed = ndst[m] - c * L
        eq = ehalf[m]
        okey = ed * 2 + eq
        order = np.argsort(okey, kind="stable")
        es, ed, eq = es[order], ed[order], eq[order]
        j = _cumcount(okey[order])

        grid_idx = np.zeros((128, Wtot), np.int64)
        grid_msk = np.zeros((128, Wtot), np.float32)
        # column offset of (block, pass) region start
        colA = {}
        colB = {}
        for b in range(NBLK):
            offA = offB = None
            for (q, w, coff, _s) in chunks[b]:
                if q == 0 and offA is None:
                    offA = coff
                if q == 1 and offB is None:
                    offB = coff
            colA[b] = offA
            colB[b] = offB
        blk = ed // 128
        p = ed % 128
        base = np.where(eq == 0,
                        np.array([colA[b] if colA[b] is not None else 0 for b in range(NBLK)])[blk],
                        np.array([colB[b] if colB[b] is not None else 0 for b in range(NBLK)])[blk])
        col = base + j
        val = np.where(eq == 0, es, es - ABOUND)
        grid_idx[p, col] = val
        grid_msk[p, col] = 1.0

        # wrap idx per chunk: stream position i = col_local*128 + p
        pieces = []
        for b in range(NBLK):
            for (q, w, coff, _s) in chunks[b]:
                g = grid_idx[:, coff : coff + w]          # [128, w]
                flat = g.T.reshape(-1)                     # i = col*128 + p
                S = (128 * w) // 16
                t = flat.reshape(S, 16).T.astype(np.int16)  # [16, S]
                tt = np.zeros((128, S), np.int16)
                for gfac in range(8):
                    tt[gfac * 16 : (gfac + 1) * 16] = t
                pieces.append(tt)
        idx_all = np.concatenate(pieces, axis=1)
        idx_alls.append(idx_all)
        msk_alls.append(grid_msk.astype(ml_dtypes.bfloat16))

    meta = dict(chunks=chunks, Wtot=Wtot, S16tot=S16tot,
                node_order=node_order, perm=perm)
    return meta, idx_alls, msk_alls


def _weights_ext(W, al, ar, heads, hd):
    K = W.shape[0]
    Wr = W.reshape(K, heads, hd)
    A = np.einsum("khd,hd->kh", Wr, al).astype(np.float32)
    B = np.einsum("khd,hd->kh", Wr, ar).astype(np.float32)
    We = np.concatenate([W, A, B], axis=1).astype(np.float32)
    pad = (-We.shape[1]) % 4
    if pad:
        We = np.concatenate([We, np.zeros((K, pad), np.float32)], axis=1)
    return We


def _build_program(meta):
    chunks = meta["chunks"]
    S16tot = meta["S16tot"]
    Wtot = meta["Wtot"]

    nc = bacc.Bacc("TRN2")
    LP = NBLK * 128  # padded node count per core (6272)

    featT = nc.dram_tensor("featT", [F0, L], F32, kind="ExternalInput")
    W1e = nc.dram_tensor("W1e", [F0, 136], F32, kind="ExternalInput")
    W2e = nc.dram_tensor("W2e", [HID, 136], F32, kind="ExternalInput")
    W3e = nc.dram_tensor("W3e", [HID, 68], F32, kind="ExternalInput")
    b1r = nc.dram_tensor("b1r", [128, HID], F32, kind="ExternalInput")
    b2r = nc.dram_tensor("b2r", [128, HID], F32, kind="ExternalInput")
    b3r = nc.dram_tensor("b3r", [128, OUT], F32, kind="ExternalInput")
    bcol_in = nc.dram_tensor("bcol", [128, 2], F32, kind="ExternalInput")
    ident_in = nc.dram_tensor("ident", [128, 128], F32, kind="ExternalInput")
    idx_in = nc.dram_tensor("idx_all", [128, S16tot], I16, kind="ExternalInput")
    msk_in = nc.dram_tensor("msk_all", [128, Wtot], BF16, kind="ExternalInput")
    out_ext = nc.dram_tensor("out", [LP, OUT], F32, kind="ExternalOutput")

    # gather row stride (ROW*) vs compact used width (RC*): the AllGather
    # ships only the used columns into a strided view of the full table.
    ROW12, ROW3 = 256, 128
    RC12, RC3 = 136, 66
    tab_loc1 = nc.dram_tensor("tab_loc1", [L, ROW12], BF16)
    tab_loc2 = nc.dram_tensor("tab_loc2", [L, ROW12], BF16)
    tab_loc3 = nc.dram_tensor("tab_loc3", [L, ROW3], BF16)
    tab1 = nc.dram_tensor("tab1", [N, ROW12], BF16, addr_space="Shared")
    tab2 = nc.dram_tensor("tab2", [N, ROW12], BF16, addr_space="Shared")
    tab3 = nc.dram_tensor("tab3", [N, ROW3], BF16, addr_space="Shared")

    layers = [
        dict(Fin=F0, Fout=HID, heads=HEADS, hd=HD, W=W1e, ncols=136, row=ROW12,
             rc=RC12, tloc=tab_loc1, tfull=tab1, brep=b1r, relu=True),
        dict(Fin=HID, Fout=HID, heads=HEADS, hd=HD, W=W2e, ncols=136, row=ROW12,
             rc=RC12, tloc=tab_loc2, tfull=tab2, brep=b2r, relu=True),
        dict(Fin=HID, Fout=OUT, heads=1, hd=OUT, W=W3e, ncols=68, row=ROW3,
             rc=RC3, tloc=tab_loc3, tfull=tab3, brep=b3r, relu=False),
    ]

    with tile.TileContext(nc) as tc:
        with (
            tc.tile_pool(name="persist", bufs=1) as pp,
            tc.tile_pool(name="work", bufs=3) as wp,
            tc.tile_pool(name="mg", bufs=4) as mgp,
            tc.tile_pool(name="psum", bufs=4, space="PSUM") as psp,
            tc.tile_pool(name="psumT", bufs=2, space="PSUM") as pspT,
        ):
            idx_sb = pp.tile([128, S16tot], I16, tag="idx")
            nc.sync.dma_start(idx_sb[:], idx_in[:])
            msk_sb = pp.tile([128, Wtot], BF16, tag="msk")
            nc.sync.dma_start(msk_sb[:], msk_in[:])
            ident = pp.tile([128, 128], F32, tag="ident")
            nc.sync.dma_start(ident[:], ident_in[:])

            # xT double buffer (features x nodes), fp32
            xT_a0 = pp.tile([128, LP], F32, tag="xTa0")
            xT_a1 = pp.tile([128, LP], F32, tag="xTa1")  # 2nd K-tile (layer 0 only)
            xT_b = pp.tile([128, LP], F32, tag="xTb")
            nc.sync.dma_start(xT_a0[:, 0:L], featT[0:128, :])
            nc.sync.dma_start(xT_a1[:, 0:L], featT[128:256, :])

            er_all = pp.tile([128, NBLK, HEADS], F32, tag="er")
            bias_sb = pp.tile([128, HID], F32, tag="bias")
            bcol_sb = pp.tile([128, 2], F32, tag="bcol")
            nc.sync.dma_start(bcol_sb[:], bcol_in[:])

            for li, lay in enumerate(layers):
                heads, hd = lay["heads"], lay["hd"]
                Fout, ncols, ROW = lay["Fout"], lay["ncols"], lay["row"]
                ktiles = lay["Fin"] // 128
                xts = [xT_a0, xT_a1][:ktiles] if li == 0 else \
                      ([xT_b] if li == 1 else [xT_a0])
                xt_next = xT_b if li == 0 else (xT_a0 if li == 1 else None)

                # weights for this layer into SBUF
                wsb = wp.tile([128, ktiles, ncols], F32, tag="wsb")
                for kt in range(ktiles):
                    nc.sync.dma_start(wsb[:, kt, :], lay["W"][kt * 128 : (kt + 1) * 128, :])
                nc.sync.dma_start(bias_sb[:, 0:Fout], lay["brep"][:, 0:Fout])

                # ---- dense phase ----
                for cb in range(NBLK):
                    n0 = cb * 128
                    nn = min(128, L - n0)
                    ps = psp.tile([128, ncols], F32, tag="dps")
                    for kt in range(ktiles):
                        nc.tensor.matmul(
                            ps[0:nn, :], xts[kt][:, n0 : n0 + nn], wsb[:, kt, :],
                            start=(kt == 0), stop=(kt == ktiles - 1))
                    row_t = wp.tile([128, ROW], BF16, tag="rowt")
                    # h -> bf16
                    nc.vector.tensor_copy(row_t[0:nn, 0:Fout], ps[0:nn, 0:Fout])
                    # el fp32 bits at bf16 cols [Fout_pad : +2*heads]
                    elo = Fout  # bf16 col offset of el (fp32 pairs)
                    nc.vector.tensor_copy(
                        row_t[0:nn, elo : elo + 2 * heads].bitcast(F32),
                        ps[0:nn, Fout : Fout + heads])
                    # er -> SBUF er_all
                    nc.vector.tensor_copy(
                        er_all[0:nn, cb, 0:heads],
                        ps[0:nn, Fout + heads : Fout + 2 * heads])
                    nc.sync.dma_start(lay["tloc"][n0 : n0 + nn, :], row_t[0:nn, :])

                # ---- allgather ----
                nc.gpsimd.collective_compute(
                    "AllGather", OP.bypass,
                    replica_groups=[list(range(NC))],
                    ins=[lay["tloc"][:]], outs=[lay["tfull"][:]])

                TQ0 = lay["tfull"][0:ABOUND, :]
                TQ1 = lay["tfull"][ABOUND:N, :]

                # ---- edge phase ----
                # mg rows are [128, w, 8|4, 32]: groups 0..heads_g-1 hold h,
                # group heads_g's first 2*heads bf16 hold el as f32 pairs.
                heads_g = Fout // 32  # 32-wide feature groups (4 or 2)
                GR = ROW // 32        # groups per gathered row (8 or 4)
                for b in range(NBLK):
                    W = sum(w for (_q, w, _c, _s) in chunks[b])
                    mbuf = wp.tile([128, heads_g, 32, W], F16, tag="mbuf")
                    exw = wp.tile([128, heads, W], F16, tag="exw")
                    erb = er_all[:, b, 0:heads]

                    def back_half(st):
                        # mask + weighted-message for a chunk whose exp is done
                        (w, coff, jo, mg, ex) = st
                        nc.vector.tensor_tensor(
                            exw[:, :, jo : jo + w],
                            ex[:].rearrange("p w h -> p h w"),
                            msk_sb[:, coff : coff + w]
                                .unsqueeze(1).broadcast_to([128, heads, w]),
                            OP.mult)
                        exb = exw[:, :, jo : jo + w].unsqueeze(2).broadcast_to(
                            [128, heads_g, 32, w])
                        nc.vector.tensor_tensor(
                            mbuf[:, :, :, jo : jo + w],
                            mg[:, :, 0:heads_g, :].rearrange("p w g d -> p g d w"),
                            exb, OP.mult)

                    pending = None
                    joff = 0
                    for (q, w, coff, soff) in chunks[b]:
                        mg = mgp.tile([128, w, GR, 32], BF16, tag="mg")
                        nidx = 128 * w
                        nc.gpsimd.dma_gather(
                            mg[:].rearrange("p w g d -> p w (g d)"),
                            TQ0 if q == 0 else TQ1,
                            idx_sb[:, soff : soff + nidx // 16],
                            nidx, nidx, ROW, single_packet=False)
                        elv = mg[:, :, heads_g, 0 : 2 * heads].bitcast(F32)
                        lg = wp.tile([128, w, heads], F32, tag="lg")
                        nc.vector.tensor_tensor(
                            lg[:], elv,
                            erb.unsqueeze(1).broadcast_to([128, w, heads]), OP.add)
                        # leaky relu on DVE: max(x, NEG*x)
                        lr = wp.tile([128, w, heads], F32, tag="lr")
                        nc.vector.scalar_tensor_tensor(
                            lr[:], lg[:], NEG, lg[:], op0=OP.mult, op1=OP.max)
                        ex = wp.tile([128, w, heads], F32, tag="ex")
                        nc.scalar.activation(ex[:], lr[:], AF.Exp)
                        if pending is not None:
                            back_half(pending)
                        pending = (w, coff, joff, mg, ex)
                        joff += w
                    back_half(pending)
                    # per-block reduction over all W edge slots
                    den = wp.tile([128, heads], F32, tag="den")
                    nc.vector.tensor_reduce(
                        den[:], exw[:, :, 0:W], axis=mybir.AxisListType.X, op=OP.add)
                    # tree-add over the W edge slots (fp16, in place on mbuf)
                    ww = W
                    while ww > 1:
                        hh = (ww + 1) // 2
                        nc.vector.tensor_tensor(
                            mbuf[:, :, :, 0 : ww - hh], mbuf[:, :, :, 0 : ww - hh],
                            mbuf[:, :, :, hh:ww], OP.add)
                        ww = hh
                    acc3 = wp.tile([128, heads_g, 32], F32, tag="acc3")
                    nc.vector.tensor_copy(acc3[:], mbuf[:, :, :, 0])
                    # normalize + bias (+relu) per block
                    nc.vector.tensor_scalar_max(den[:], den[:], 1e-30)
                    rden = wp.tile([128, heads], F32, tag="rden")
                    nc.vector.reciprocal(rden[:], den[:])
                    accF = wp.tile([128, Fout], F32, tag="accF")
                    dg = heads_g // heads
                    for g in range(heads_g):
                        nc.scalar.activation(
                            accF[:, g * 32 : (g + 1) * 32], acc3[:, g, :],
                            AF.Copy, scale=rden[:, g // dg : g // dg + 1])
                    if lay["relu"]:
                        # transpose, then relu(x + bias) with per-feature bias
                        # (features sit on partitions after the transpose)
                        pst = pspT.tile([128, 128], F32, tag="tps")
                        nc.tensor.transpose(pst[:], accF[:], ident[:])
                        nc.scalar.activation(
                            xt_next[:, b * 128 : (b + 1) * 128], pst[:],
                            AF.Relu, bias=bcol_sb[:, li : li + 1])
                    else:
                        nc.vector.tensor_add(accF[:], accF[:], bias_sb[:, 0:Fout])
                        nc.sync.dma_start(out_ext[b * 128 : (b + 1) * 128, :], accF[:, 0:OUT])

    _split_multiwaits(nc)
    nc.compile()
    return nc


_CACHE = {}
LAST_EXEC_NS = None


def kernel(feat, src, dst, W1, al1, ar1, b1, W2, al2, ar2, b2, W3, al3, ar3, b3):
    feat = np.asarray(feat, np.float32)
    key = (int(np.asarray(src[:100]).sum()), int(np.asarray(dst[:100]).sum()))
    if key in _CACHE:
        nc, meta, idx_alls, msk_alls = _CACHE[key]
    else:
        meta, idx_alls, msk_alls = _preprocess(src, dst)
        nc = _build_program(meta)
        _CACHE[key] = (nc, meta, idx_alls, msk_alls)

    node_order = meta["node_order"]

    W1e = _weights_ext(np.asarray(W1, np.float32), np.asarray(al1, np.float32),
                       np.asarray(ar1, np.float32), HEADS, HD)
    W2e = _weights_ext(np.asarray(W2, np.float32), np.asarray(al2, np.float32),
                       np.asarray(ar2, np.float32), HEADS, HD)
    W3e = _weights_ext(np.asarray(W3, np.float32), np.asarray(al3, np.float32),
                       np.asarray(ar3, np.float32), 1, OUT)
    assert W1e.shape[1] == 136 and W3e.shape[1] == 68

    ident = np.eye(128, dtype=np.float32)
    b1r = np.tile(np.asarray(b1, np.float32)[None, :], (128, 1))
    b2r = np.tile(np.asarray(b2, np.float32)[None, :], (128, 1))
    b3r = np.tile(np.asarray(b3, np.float32)[None, :], (128, 1))
    bcol = np.stack([np.asarray(b1, np.float32),
                     np.asarray(b2, np.float32)], axis=1)

    in_maps = []
    for c in range(NC):
        nodes = node_order[c * L : (c + 1) * L]
        featT_c = np.ascontiguousarray(feat[nodes, :].T)
        in_maps.append(dict(
            featT=featT_c, W1e=W1e, W2e=W2e, W3e=W3e,
            b1r=b1r, b2r=b2r, b3r=b3r, bcol=bcol, ident=ident,
            idx_all=idx_alls[c], msk_all=np.asarray(msk_alls[c]),
        ))

    kw = {}
    if os.environ.get("GAT_TRACE"):
        kw = dict(trace=True, tmpdir=os.environ.get("GAT_TRACE_DIR") or None)
    res = run_bass_kernel_spmd(nc, in_maps, list(range(NC)), **kw)
    global LAST_EXEC_NS
    if getattr(res, "exec_time_ns", None):
        LAST_EXEC_NS = res.exec_time_ns

    out = np.empty((N, OUT), np.float32)
    for c in range(NC):
        nodes = node_order[c * L : (c + 1) * L]
        out[nodes] = res.results[c]["out"][0:L, :]
    return out



# revision 35
# speedup vs baseline: 1.3395x; 1.3395x over previous
"""GAT (3-layer DGL-style) on 8 Trainium2 NeuronCores.

Sharding: nodes partitioned contiguously across 8 cores (6250 each, relabeled
within each core by in-degree for slot-grid uniformity). Edges sharded by dst
core. Per layer: distributed dense matmul produces per-node rows
[h | el | er], AllGather replicates the row table to every core, then each
core runs the edge phase (gather by src via dma_gather, per-dst softmax in a
[dst-partition x slot] layout, weighted accumulation) for its own dsts.
"""

import os

import numpy as np
import ml_dtypes

import concourse.bacc as bacc
import concourse.bass as bass
import concourse.mybir as mybir
from concourse import tile
from concourse._compat import cdiv
from concourse.bass_utils import run_bass_kernel_spmd
from bass_rust import SemaphoreHandle

N = 50000
E = 800000
NC = 8
L = N // NC              # 6250 nodes per core
NBLK = cdiv(L, 128)      # 49 dst blocks per core
HEADS = 4
HD = 32
HID = 128
OUT = 64
F0 = 256
NEG = 0.2
CH = 16                  # max slots per gather chunk
ABOUND = 5 * L           # nodes with new id < ABOUND are "pass A" (31250)

F32 = mybir.dt.float32
BF16 = mybir.dt.bfloat16
F16 = mybir.dt.float16
I16 = mybir.dt.int16
AF = mybir.ActivationFunctionType
OP = mybir.AluOpType


def _split_multiwaits(nc):
    nsplit = 0
    for bb in nc.main_func.blocks:
        i = 0
        while i < len(bb.instructions):
            ins = bb.instructions[i]
            si = ins.sync_info
            if si is not None and si.on_wait and len(si.on_wait) > 1:
                waits = list(si.on_wait)
                new_insts = []
                for w in waits[:-1]:
                    h = SemaphoreHandle(name=w.ant_name, num=w.id)
                    eng = nc.engines[ins.engine]
                    if w.wait_mode == "sem-ge-imm":
                        wi = eng.wait_ge(h, w.wait_value)
                    elif w.wait_mode == "sem-eq-imm":
                        wi = eng.wait_op(h, w.wait_value, "==")
                    else:
                        raise AssertionError(w.wait_mode)
                    removed = False
                    for b2 in nc.main_func.blocks:
                        if b2.instructions and b2.instructions[-1].name == wi.ins.name:
                            b2.instructions.pop()
                            removed = True
                            break
                    assert removed
                    new_insts.append(wi.ins)
                si.on_wait = [waits[-1]]
                for k, n in enumerate(new_insts):
                    bb.instructions.insert(i + k, n)
                i += len(new_insts)
                nsplit += 1
            i += 1
    return nsplit


def _cumcount(groups):
    """j-th occurrence index within each group (groups sorted)."""
    n = len(groups)
    if n == 0:
        return np.zeros(0, np.int64)
    first = np.r_[True, groups[1:] != groups[:-1]]
    idx = np.arange(n)
    start = idx[first]
    return idx - np.repeat(start, np.diff(np.r_[idx[first], n]))


def _preprocess(src, dst):
    src = np.asarray(src, np.int64)
    dst = np.asarray(dst, np.int64)
    half = (src // L) >= 5          # pass B edges (src in cores 5-7)

    degA = np.bincount(dst[~half], minlength=N)
    degB = np.bincount(dst[half], minlength=N)

    perm = np.empty(N, np.int64)        # old id -> new id
    node_order = np.empty(N, np.int64)  # new id -> old id
    for c in range(NC):
        nodes = np.arange(c * L, (c + 1) * L)
        order = np.lexsort((-degB[nodes], -(degA[nodes] // 3)))
        node_order[c * L : (c + 1) * L] = nodes[order]
        perm[nodes[order]] = c * L + np.arange(L)

    nsrc = perm[src]
    ndst = perm[dst]
    ehalf = (nsrc >= ABOUND).astype(np.int64)

    # per-(core, block, pass) slot grids
    # counts per (dst, pass)
    cntA = np.bincount(ndst[ehalf == 0], minlength=N)
    cntB = np.bincount(ndst[ehalf == 1], minlength=N)

    # program-level W per (block, pass): max over cores
    WA = np.zeros(NBLK, np.int64)
    WB = np.zeros(NBLK, np.int64)
    for c in range(NC):
        la = cntA[c * L : (c + 1) * L]
        lb = cntB[c * L : (c + 1) * L]
        pa = np.zeros(NBLK * 128, np.int64)
        pb = np.zeros(NBLK * 128, np.int64)
        pa[:L] = la
        pb[:L] = lb
        WA = np.maximum(WA, pa.reshape(NBLK, 128).max(1))
        WB = np.maximum(WB, pb.reshape(NBLK, 128).max(1))

    # chunk lists per block: [(pass, width, col_off, idx_off16)]
    def split_w(w):
        out = []
        while w > 0:
            t = min(CH, w)
            out.append(t)
            w -= t
        return out

    chunks = []        # per block: list of (q, w)
    Wtot = 0
    S16tot = 0
    for b in range(NBLK):
        cl = []
        for q, Wq in ((0, WA[b]), (1, WB[b])):
            for w in split_w(int(Wq)):
                cl.append((q, w, Wtot, S16tot))
                Wtot += w
                S16tot += (128 * w) // 16
        chunks.append(cl)

    # per-core grids
    # edge order: by (block, pass, partition), j = occurrence rank
    idx_alls = []
    msk_alls = []
    for c in range(NC):
        m = (ndst // L) == c
        es = nsrc[m]
        ed = ndst[m] - c * L
        eq = ehalf[m]
        okey = ed * 2 + eq
        order = np.argsort(okey, kind="stable")
        es, ed, eq = es[order], ed[order], eq[order]
        j = _cumcount(okey[order])

        grid_idx = np.zeros((128, Wtot), np.int64)
        grid_msk = np.zeros((128, Wtot), np.float32)
        # column offset of (block, pass) region start
        colA = {}
        colB = {}
        for b in range(NBLK):
            offA = offB = None
            for (q, w, coff, _s) in chunks[b]:
                if q == 0 and offA is None:
                    offA = coff
                if q == 1 and offB is None:
                    offB = coff
            colA[b] = offA
            colB[b] = offB
        blk = ed // 128
        p = ed % 128
        base = np.where(eq == 0,
                        np.array([colA[b] if colA[b] is not None else 0 for b in range(NBLK)])[blk],
                        np.array([colB[b] if colB[b] is not None else 0 for b in range(NBLK)])[blk])
        col = base + j
        val = np.where(eq == 0, es, es - ABOUND)
        grid_idx[p, col] = val
        grid_msk[p, col] = 1.0

        # wrap idx per chunk: stream position i = col_local*128 + p
        pieces = []
        for b in range(NBLK):
            for (q, w, coff, _s) in chunks[b]:
                g = grid_idx[:, coff : coff + w]          # [128, w]
                flat = g.T.reshape(-1)                     # i = col*128 + p
                S = (128 * w) // 16
                t = flat.reshape(S, 16).T.astype(np.int16)  # [16, S]
                tt = np.zeros((128, S), np.int16)
                for gfac in range(8):
                    tt[gfac * 16 : (gfac + 1) * 16] = t
                pieces.append(tt)
        idx_all = np.concatenate(pieces, axis=1)
        idx_alls.append(idx_all)
        msk_alls.append(grid_msk.astype(ml_dtypes.bfloat16))

    meta = dict(chunks=chunks, Wtot=Wtot, S16tot=S16tot,
                node_order=node_order, perm=perm)
    return meta, idx_alls, msk_alls


def _weights_ext(W, al, ar, heads, hd):
    K = W.shape[0]
    Wr = W.reshape(K, heads, hd)
    A = np.einsum("khd,hd->kh", Wr, al).astype(np.float32)
    B = np.einsum("khd,hd->kh", Wr, ar).astype(np.float32)
    We = np.concatenate([W, A, B], axis=1).astype(np.float32)
    pad = (-We.shape[1]) % 4
    if pad:
        We = np.concatenate([We, np.zeros((K, pad), np.float32)], axis=1)
    return We


def _build_program(meta):
    chunks = meta["chunks"]
    S16tot = meta["S16tot"]
    Wtot = meta["Wtot"]

    nc = bacc.Bacc("TRN2")
    LP = NBLK * 128  # padded node count per core (6272)

    featT = nc.dram_tensor("featT", [F0, L], BF16, kind="ExternalInput")
    W1e = nc.dram_tensor("W1e", [F0, 136], BF16, kind="ExternalInput")
    W2e = nc.dram_tensor("W2e", [HID, 136], BF16, kind="ExternalInput")
    W3e = nc.dram_tensor("W3e", [HID, 68], BF16, kind="ExternalInput")
    b1r = nc.dram_tensor("b1r", [128, HID], F32, kind="ExternalInput")
    b2r = nc.dram_tensor("b2r", [128, HID], F32, kind="ExternalInput")
    b3r = nc.dram_tensor("b3r", [128, OUT], F32, kind="ExternalInput")
    bcol_in = nc.dram_tensor("bcol", [128, 2], F32, kind="ExternalInput")
    ident_in = nc.dram_tensor("ident", [128, 128], F32, kind="ExternalInput")
    idx_in = nc.dram_tensor("idx_all", [128, S16tot], I16, kind="ExternalInput")
    msk_in = nc.dram_tensor("msk_all", [128, Wtot], BF16, kind="ExternalInput")
    out_ext = nc.dram_tensor("out", [LP, OUT], F32, kind="ExternalOutput")

    # gather row stride (ROW*) vs compact used width (RC*): the AllGather
    # ships only the used columns into a strided view of the full table.
    ROW12, ROW3 = 256, 128
    RC12, RC3 = 136, 66
    tab_loc1 = nc.dram_tensor("tab_loc1", [L, ROW12], BF16)
    tab_loc2 = nc.dram_tensor("tab_loc2", [L, ROW12], BF16)
    tab_loc3 = nc.dram_tensor("tab_loc3", [L, RC3], BF16)
    tab1 = nc.dram_tensor("tab1", [N, ROW12], BF16, addr_space="Shared")
    tab2 = nc.dram_tensor("tab2", [N, ROW12], BF16, addr_space="Shared")
    tab3 = nc.dram_tensor("tab3", [N, ROW3], BF16, addr_space="Shared")
    tab3c = nc.dram_tensor("tab3c", [N, RC3], BF16, addr_space="Shared")

    layers = [
        dict(Fin=F0, Fout=HID, heads=HEADS, hd=HD, W=W1e, ncols=136, row=ROW12,
             rc=RC12, tloc=tab_loc1, tfull=tab1, brep=b1r, relu=True),
        dict(Fin=HID, Fout=HID, heads=HEADS, hd=HD, W=W2e, ncols=136, row=ROW12,
             rc=RC12, tloc=tab_loc2, tfull=tab2, brep=b2r, relu=True),
        dict(Fin=HID, Fout=OUT, heads=1, hd=OUT, W=W3e, ncols=68, row=ROW3,
             rc=RC3, tloc=tab_loc3, tfull=tab3, brep=b3r, relu=False),
    ]

    with tile.TileContext(nc) as tc:
        with (
            tc.tile_pool(name="persist", bufs=1) as pp,
            tc.tile_pool(name="work", bufs=3) as wp,
            tc.tile_pool(name="mg", bufs=4) as mgp,
            tc.tile_pool(name="psum", bufs=4, space="PSUM") as psp,
            tc.tile_pool(name="psumT", bufs=2, space="PSUM") as pspT,
        ):
            idx_sb = pp.tile([128, S16tot], I16, tag="idx")
            nc.sync.dma_start(idx_sb[:], idx_in[:])
            msk_sb = pp.tile([128, Wtot], BF16, tag="msk")
            nc.sync.dma_start(msk_sb[:], msk_in[:])
            ident = pp.tile([128, 128], F32, tag="ident")
            nc.sync.dma_start(ident[:], ident_in[:])

            # xT double buffer (features x nodes), fp32
            xT_a0 = pp.tile([128, LP], BF16, tag="xTa0")
            xT_a1 = pp.tile([128, LP], BF16, tag="xTa1")  # 2nd K-tile (layer 0 only)
            xT_b = pp.tile([128, LP], BF16, tag="xTb")
            nc.sync.dma_start(xT_a0[:, 0:L], featT[0:128, :])
            nc.sync.dma_start(xT_a1[:, 0:L], featT[128:256, :])

            er_tiles = [pp.tile([128, NBLK, HEADS], F32, tag="er0"),
                        pp.tile([128, NBLK, HEADS], F32, tag="er1")]
            bias_sb = pp.tile([128, HID], F32, tag="bias")
            nc.sync.dma_start(bias_sb[:, 0:OUT], b3r[:, 0:OUT])
            bcol_sb = pp.tile([128, 2], F32, tag="bcol")
            nc.sync.dma_start(bcol_sb[:], bcol_in[:])

            wsbs = {}

            def load_wsb(li2):
                lay2 = layers[li2]
                kt2 = lay2["Fin"] // 128
                w2 = wp.tile([128, kt2, lay2["ncols"]], BF16, tag="wsb")
                for kt in range(kt2):
                    nc.sync.dma_start(
                        w2[:, kt, :], lay2["W"][kt * 128 : (kt + 1) * 128, :])
                wsbs[li2] = w2

            def dense_block(li2, cb):
                lay2 = layers[li2]
                Fout2, ncols2 = lay2["Fout"], lay2["ncols"]
                heads2 = lay2["heads"]
                kt2 = lay2["Fin"] // 128
                xts2 = [xT_a0, xT_a1][:kt2] if li2 == 0 else \
                       ([xT_b] if li2 == 1 else [xT_a0])
                ert = er_tiles[li2 % 2]
                n0 = cb * 128
                nn = min(128, L - n0)
                ps = psp.tile([128, ncols2], F32, tag="dps")
                for kt in range(kt2):
                    nc.tensor.matmul(
                        ps[0:nn, :], xts2[kt][:, n0 : n0 + nn], wsbs[li2][:, kt, :],
                        start=(kt == 0), stop=(kt == kt2 - 1))
                row_t = wp.tile([128, lay2["row"] if li2 < 2 else RC3], BF16,
                                tag="rowt")
                # h -> bf16, el fp32 bits at bf16 cols [Fout : Fout+2*heads]
                nc.vector.tensor_copy(row_t[0:nn, 0:Fout2], ps[0:nn, 0:Fout2])
                nc.vector.tensor_copy(
                    row_t[0:nn, Fout2 : Fout2 + 2 * heads2].bitcast(F32),
                    ps[0:nn, Fout2 : Fout2 + heads2])
                nc.vector.tensor_copy(
                    ert[0:nn, cb, 0:heads2],
                    ps[0:nn, Fout2 + heads2 : Fout2 + 2 * heads2])
                nc.sync.dma_start(lay2["tloc"][n0 : n0 + nn, :], row_t[0:nn, :])

            # layer-0 dense phase (cannot overlap anything earlier)
            load_wsb(0)
            for cb in range(NBLK):
                dense_block(0, cb)

            for li, lay in enumerate(layers):
                heads, hd = lay["heads"], lay["hd"]
                Fout, ncols, ROW = lay["Fout"], lay["ncols"], lay["row"]
                xt_next = xT_b if li == 0 else (xT_a0 if li == 1 else None)
                er_all = er_tiles[li % 2]

                # ---- allgather ----
                if li < 2:
                    nc.gpsimd.collective_compute(
                        "AllGather", OP.bypass,
                        replica_groups=[list(range(NC))],
                        ins=[lay["tloc"][:]], outs=[lay["tfull"][:]])
                else:
                    # layer 3 rows are only 66/128 elems used: gather the
                    # compact columns, then expand locally into the strided
                    # 128-elem-pitch gather table.
                    nc.gpsimd.collective_compute(
                        "AllGather", OP.bypass,
                        replica_groups=[list(range(NC))],
                        ins=[lay["tloc"][:]], outs=[tab3c[:]])
                    nc.sync.dma_start(lay["tfull"][0:N, 0:RC3], tab3c[:])

                TQ0 = lay["tfull"][0:ABOUND, :]
                TQ1 = lay["tfull"][ABOUND:N, :]

                # ---- edge phase ----
                # mg rows are [128, w, 8|4, 32]: groups 0..heads_g-1 hold h,
                # group heads_g's first 2*heads bf16 hold el as f32 pairs.
                heads_g = Fout // 32  # 32-wide feature groups (4 or 2)
                GR = ROW // 32        # groups per gathered row (8 or 4)
                for b in range(NBLK):
                    W = sum(w for (_q, w, _c, _s) in chunks[b])
                    mbuf = wp.tile([128, heads_g, 32, W], F16, tag="mbuf")
                    exw = wp.tile([128, heads, W], F16, tag="exw")
                    erb = er_all[:, b, 0:heads]

                    def back_half(st):
                        # mask + weighted-message for a chunk whose exp is done
                        (w, coff, jo, mg, ex) = st
                        nc.vector.tensor_tensor(
                            exw[:, :, jo : jo + w],
                            ex[:].rearrange("p w h -> p h w"),
                            msk_sb[:, coff : coff + w]
                                .unsqueeze(1).broadcast_to([128, heads, w]),
                            OP.mult)
                        exb = exw[:, :, jo : jo + w].unsqueeze(2).broadcast_to(
                            [128, heads_g, 32, w])
                        nc.vector.tensor_tensor(
                            mbuf[:, :, :, jo : jo + w],
                            mg[:, :, 0:heads_g, :].rearrange("p w g d -> p g d w"),
                            exb, OP.mult)

                    pending = None
                    joff = 0
                    for (q, w, coff, soff) in chunks[b]:
                        mg = mgp.tile([128, w, GR, 32], BF16, tag="mg")
                        nidx = 128 * w
                        nc.gpsimd.dma_gather(
                            mg[:].rearrange("p w g d -> p w (g d)"),
                            TQ0 if q == 0 else TQ1,
                            idx_sb[:, soff : soff + nidx // 16],
                            nidx, nidx, ROW, single_packet=False)
                        elv = mg[:, :, heads_g, 0 : 2 * heads].bitcast(F32)
                        lg = wp.tile([128, w, heads], F32, tag="lg")
                        nc.vector.tensor_tensor(
                            lg[:], elv,
                            erb.unsqueeze(1).broadcast_to([128, w, heads]), OP.add)
                        # leaky relu on DVE: max(x, NEG*x)
                        lr = wp.tile([128, w, heads], F32, tag="lr")
                        nc.vector.scalar_tensor_tensor(
                            lr[:], lg[:], NEG, lg[:], op0=OP.mult, op1=OP.max)
                        ex = wp.tile([128, w, heads], F32, tag="ex")
                        nc.scalar.activation(ex[:], lr[:], AF.Exp)
                        if pending is not None:
                            back_half(pending)
                        pending = (w, coff, joff, mg, ex)
                        joff += w
                    back_half(pending)
                    # per-block reduction over all W edge slots
                    den = wp.tile([128, heads], F32, tag="den")
                    nc.vector.tensor_reduce(
                        den[:], exw[:, :, 0:W], axis=mybir.AxisListType.X, op=OP.add)
                    # tree-add over the W edge slots (fp16, in place on mbuf)
                    ww = W
                    while ww > 1:
                        hh = (ww + 1) // 2
                        nc.vector.tensor_tensor(
                            mbuf[:, :, :, 0 : ww - hh], mbuf[:, :, :, 0 : ww - hh],
                            mbuf[:, :, :, hh:ww], OP.add)
                        ww = hh
                    acc3 = wp.tile([128, heads_g, 32], F32, tag="acc3")
                    nc.vector.tensor_copy(acc3[:], mbuf[:, :, :, 0])
                    # normalize + bias (+relu) per block
                    nc.vector.tensor_scalar_max(den[:], den[:], 1e-30)
                    rden = wp.tile([128, heads], F32, tag="rden")
                    nc.vector.reciprocal(rden[:], den[:])
                    accF = wp.tile([128, Fout], F32, tag="accF")
                    dg = heads_g // heads
                    for g in range(heads_g):
                        nc.scalar.activation(
                            accF[:, g * 32 : (g + 1) * 32], acc3[:, g, :],
                            AF.Copy, scale=rden[:, g // dg : g // dg + 1])
                    if lay["relu"]:
                        # transpose, then relu(x + bias) with per-feature bias
                        # (features sit on partitions after the transpose)
                        pst = pspT.tile([128, 128], F32, tag="tps")
                        nc.tensor.transpose(pst[:], accF[:], ident[:])
                        nc.scalar.activation(
                            xt_next[:, b * 128 : (b + 1) * 128], pst[:],
                            AF.Relu, bias=bcol_sb[:, li : li + 1])
                    else:
                        nc.vector.tensor_add(accF[:], accF[:], bias_sb[:, 0:Fout])
                        nc.sync.dma_start(out_ext[b * 128 : (b + 1) * 128, :], accF[:, 0:OUT])

    _split_multiwaits(nc)
    nc.compile()
    return nc


_CACHE = {}
LAST_EXEC_NS = None


def kernel(feat, src, dst, W1, al1, ar1, b1, W2, al2, ar2, b2, W3, al3, ar3, b3):
    feat = np.asarray(feat, np.float32)
    key = (int(np.asarray(src[:100]).sum()), int(np.asarray(dst[:100]).sum()))
    if key in _CACHE:
        nc, meta, idx_alls, msk_alls = _CACHE[key]
    else:
        meta, idx_alls, msk_alls = _preprocess(src, dst)
        nc = _build_program(meta)
        _CACHE[key] = (nc, meta, idx_alls, msk_alls)

    node_order = meta["node_order"]

    W1e = _weights_ext(np.asarray(W1, np.float32), np.asarray(al1, np.float32),
                       np.asarray(ar1, np.float32), HEADS, HD)
    W2e = _weights_ext(np.asarray(W2, np.float32), np.asarray(al2, np.float32),
                       np.asarray(ar2, np.float32), HEADS, HD)
    W3e = _weights_ext(np.asarray(W3, np.float32), np.asarray(al3, np.float32),
                       np.asarray(ar3, np.float32), 1, OUT)
    assert W1e.shape[1] == 136 and W3e.shape[1] == 68
    W1e = W1e.astype(ml_dtypes.bfloat16)
    W2e = W2e.astype(ml_dtypes.bfloat16)
    W3e = W3e.astype(ml_dtypes.bfloat16)

    ident = np.eye(128, dtype=np.float32)
    b1r = np.tile(np.asarray(b1, np.float32)[None, :], (128, 1))
    b2r = np.tile(np.asarray(b2, np.float32)[None, :], (128, 1))
    b3r = np.tile(np.asarray(b3, np.float32)[None, :], (128, 1))
    bcol = np.stack([np.asarray(b1, np.float32),
                     np.asarray(b2, np.float32)], axis=1)

    in_maps = []
    for c in range(NC):
        nodes = node_order[c * L : (c + 1) * L]
        featT_c = np.ascontiguousarray(feat[nodes, :].T).astype(ml_dtypes.bfloat16)
        in_maps.append(dict(
            featT=featT_c, W1e=W1e, W2e=W2e, W3e=W3e,
            b1r=b1r, b2r=b2r, b3r=b3r, bcol=bcol, ident=ident,
            idx_all=idx_alls[c], msk_all=np.asarray(msk_alls[c]),
        ))

    kw = {}
    if os.environ.get("GAT_TRACE"):
        kw = dict(trace=True, tmpdir=os.environ.get("GAT_TRACE_DIR") or None)
    res = run_bass_kernel_spmd(nc, in_maps, list(range(NC)), **kw)
    global LAST_EXEC_NS
    if getattr(res, "exec_time_ns", None):
        LAST_EXEC_NS = res.exec_time_ns

    out = np.empty((N, OUT), np.float32)
    for c in range(NC):
        nodes = node_order[c * L : (c + 1) * L]
        out[nodes] = res.results[c]["out"][0:L, :]
    return out

